# revision 9
# baseline (speedup 1.0000x reference)
"""Trainium2 Bass kernel for nn_Discriminator (GNN message passing), v2.

Model (see reference):
    x        = concat(normal, extreme)                     [N, 512]
    neigh    = segment_mean(x[src], dst, N)                [N, 512]
    x_gnn    = relu(neigh @ W_l + b_l + x @ W_r)           [N, 1024]
    x_mlp    = relu(x @ W_fc1 + b_fc1)                     [N, 1024]
    comb     = x_gnn + x_mlp
    gf       = segment_mean(comb, batch, G)                [64, 1024]
    out      = sigmoid(gf @ W_out + b_out)                 [64, 1]

Strategy (v2): nodes sharded by DST across 8 cores (8192 each). The edge
gather x[src] is fully determined by the (host-known) edge index, so the
host pre-expands it: for each core, edges are sorted into per-dst-block
chunks of 128 and the rows x[src]*(1/deg[dst]) are written (fp8 e4m3)
into a dense stream GM[128, NCH, 640] - columns 0:512 hold the scaled
edge features, columns 512:640 hold the one-hot dst-within-block matrix.
The device just streams GM with large sequential DMAs (no gpsimd gather,
no per-edge descriptors) and does the segment-mean as one-hot matmuls in
fp8 DoubleRow mode (K=256 per matmul, 2x feed rate).

Per 128-node block: agg PSUM [node, feat] -> fp8 copy -> 4 PE transposes
-> nmT [feat, node]. Per group of 4 blocks (512 nodes): dense matmuls in
"transposed" orientation out^T[hid, node] with the fp8 weights stationary
and nmT/xT moving (DoubleRow, K=256): a = Wl.nmT + Wr.xT, b = Wf.xT.
Bias+relu fuse into one scalar-engine activation (bias is per-partition
in this orientation). The graph readout contracts hidden with W_out
on-chip: s[node] = relu(a).W_out + relu(b).W_out via M=1 matmuls
(DoubleRow over hid pairs), so only 8192 scalars leave each core. Host
un-permutes s, does the tiny per-graph mean + sigmoid (64 values).

Blocks are processed in per-core descending-edge-count order, with the
per-rank chunk count maxed across cores, so the SPMD program is identical
on all cores while padding stays ~3%.
"""

import numpy as np
import ml_dtypes

import concourse.bass as bass
import concourse.mybir as mybir
import concourse.tile as tile
from concourse.bass_utils import run_bass_kernel_spmd
from concourse.library_overlay import lower_extended_insts

N_NODES = 65536
N_EDGES = 1048576
D2 = 512              # concat feature dim
HID = 1024
N_GRAPHS = 64
N_CORES = 8
NPC = N_NODES // N_CORES      # nodes per core
NBLK = NPC // 128             # 128-node blocks (ranks) per core
P = 128
GW = 640                      # GM row width: 512 feat + 128 one-hot
FP8 = mybir.dt.float8e4
FP32 = mybir.dt.float32
BF16 = mybir.dt.bfloat16
DR = mybir.MatmulPerfMode.DoubleRow

_NP_FP8 = ml_dtypes.float8_e4m3fn
FP8_MAX = 240.0               # TRN fp8e4 saturates at +-240 (not OCP 448)


def _legalize_multiwait(nc):
    """This container's walrus accepts at most one sync-wait per
    instruction; hoist extra waits onto standalone same-engine
    InstEventSemaphore instructions (queues are in-order, so this is
    semantically identical)."""
    n = 0
    for f in nc.m.functions:
        for blk in f.blocks:
            out = []
            changed = False
            for inst in blk.instructions:
                si = getattr(inst, "sync_info", None)
                if si is not None and len(si.on_wait) > 1:
                    waits = list(si.on_wait)
                    for w in waits[:-1]:
                        es = mybir.InstEventSemaphore(
                            name=f"mwz-{inst.name}-{n}", ins=[], outs=[])
                        n += 1
                        es.engine = inst.engine
                        es.sync_info = mybir.SyncInfo(on_wait=[w], on_update=[])
                        out.append(es)
                    inst.sync_info = mybir.SyncInfo(
                        on_wait=[waits[-1]], on_update=list(si.on_update))
                    changed = True
                out.append(inst)
            if changed:
                blk.instructions = out
    return n


def _build_program(ch_list, legalize=True):
    """Build the per-core Bass/Tile program. ch_list[r] = chunk count of
    the rank-r block (identical across cores)."""
    from contextlib import ExitStack

    ch_list = [max(int(c), 1) for c in ch_list]
    CH_MAX = max(ch_list)
    offs = np.concatenate([[0], np.cumsum(ch_list)]).astype(int)
    NCH = int(offs[-1])

    nc = bass.Bass(num_swdge_queues=1)
    GM = nc.declare_dram_parameter("GM", [P, NCH, GW], FP8, isOutput=False)
    XT = nc.declare_dram_parameter("XT", [P, 4, NPC], FP8, isOutput=False)
    WL = nc.declare_dram_parameter("WL", [P, 4, HID], FP8, isOutput=False)
    WR = nc.declare_dram_parameter("WR", [P, 4, HID], FP8, isOutput=False)
    WF = nc.declare_dram_parameter("WF", [P, 4, HID], FP8, isOutput=False)
    BL = nc.declare_dram_parameter("BL", [P, 8], FP32, isOutput=False)
    BF = nc.declare_dram_parameter("BF", [P, 8], FP32, isOutput=False)
    WO = nc.declare_dram_parameter("WO", [P, 2, 16], FP8, isOutput=False)
    IDT = nc.declare_dram_parameter("IDT", [P, P], BF16, isOutput=False)
    S_OUT = nc.declare_dram_parameter("s_out", [1, NPC], FP32, isOutput=True)

    ADD = mybir.AluOpType.add
    MAX = mybir.AluOpType.max
    RELU = mybir.ActivationFunctionType.Relu

    with ExitStack() as ctx:
        tc = ctx.enter_context(tile.TileContext(nc))
        const = ctx.enter_context(tc.tile_pool(name="const", bufs=1))
        gmpool = ctx.enter_context(tc.tile_pool(name="gm", bufs=3))
        nmpool = ctx.enter_context(tc.tile_pool(name="nm", bufs=3))
        ntpool = ctx.enter_context(tc.tile_pool(name="nmT", bufs=2))
        rpool = ctx.enter_context(tc.tile_pool(name="r", bufs=2))
        p_agg = ctx.enter_context(tc.tile_pool(name="pagg", bufs=2, space="PSUM"))
        p_tr = ctx.enter_context(tc.tile_pool(name="ptr", bufs=2, space="PSUM"))
        p_mm = ctx.enter_context(tc.tile_pool(name="pmm", bufs=3, space="PSUM"))
        p_pool = ctx.enter_context(tc.tile_pool(name="ppool", bufs=1, space="PSUM"))

        xt_sb = const.tile([P, 4, NPC], FP8, tag="xt")
        nc.scalar.dma_start(xt_sb[:], XT[:])
        wl_sb = const.tile([P, 4, HID], FP8, tag="wl")
        nc.scalar.dma_start(wl_sb[:], WL[:])
        wr_sb = const.tile([P, 4, HID], FP8, tag="wr")
        nc.scalar.dma_start(wr_sb[:], WR[:])
        wf_sb = const.tile([P, 4, HID], FP8, tag="wf")
        nc.scalar.dma_start(wf_sb[:], WF[:])
        bl_sb = const.tile([P, 8], FP32, tag="bl")
        nc.scalar.dma_start(bl_sb[:], BL[:])
        bf_sb = const.tile([P, 8], FP32, tag="bf")
        nc.scalar.dma_start(bf_sb[:], BF[:])
        wo_sb = const.tile([P, 2, 16], FP8, tag="wo")
        nc.scalar.dma_start(wo_sb[:], WO[:])
        ident = const.tile([P, P], BF16, tag="ident")
        nc.scalar.dma_start(ident[:], IDT[:])
        s_all = const.tile([1, NPC], FP32, tag="sall")

        nmT = None
        nm_q = []     # blocks aggregated but not yet transposed
        for r in range(NBLK + 1):
            if r < NBLK:
                ch = ch_list[r]
                off = int(offs[r])
                gm = gmpool.tile([P, CH_MAX, GW], FP8, tag="gm")
                # sync ring is dedicated to the gm stream; constants and
                # compute share the scalar engine
                nc.sync.dma_start(gm[:, :ch, :], GM[:, off:off + ch, :])

                # segment-sum of pre-scaled edge rows -> neigh mean [node, feat]
                agg = p_agg.tile([P, D2], FP32, tag="agg")
                npairs = ch // 2
                for j in range(npairs):
                    nc.tensor.matmul(
                        agg[:], lhsT=gm[:, 2 * j:2 * j + 2, D2:GW],
                        rhs=gm[:, 2 * j:2 * j + 2, 0:D2],
                        start=(j == 0), stop=(j == npairs - 1 and ch % 2 == 0),
                        perf_mode=DR)
                if ch % 2:
                    nc.tensor.matmul(
                        agg[:], lhsT=gm[:, ch - 1, D2:GW], rhs=gm[:, ch - 1, 0:D2],
                        start=(ch == 1), stop=True)
                nm = nmpool.tile([P, D2], BF16, tag="nm")
                nc.scalar.copy(nm[:], agg[:])
                nm_q.append((r, nm))

            # transposes lag one block so the PSUM->SBUF nm copy hides
            # under the next block's aggregation matmuls
            if len(nm_q) > 1 or r == NBLK:
                r0, nm0 = nm_q.pop(0)
                b = r0 % 4
                if b == 0:
                    nmT = ntpool.tile([P, 4, 512], FP8, tag="nmT")
                # transpose [node, feat] -> [feat, node], bf16 through PSUM
                # (fp8 transpose is rejected by walrus: needs out elem step 2)
                tr = p_tr.tile([P, 4, 256], BF16, tag="tr")
                for fs in range(4):
                    nc.tensor.transpose(tr[:, fs, 0:P],
                                        nm0[:, fs * P:(fs + 1) * P], ident[:])
                nc.vector.tensor_copy(nmT[:, :, b * P:(b + 1) * P], tr[:, :, 0:P])

                if b == 3:
                    g = r0 // 4
                    n0 = g * 512
                    ra = rpool.tile([P, 8, 512], FP8, tag="ra")
                    rb = rpool.tile([P, 8, 512], FP8, tag="rb")
                    for ht in range(8):
                        hs = slice(ht * P, (ht + 1) * P)
                        # b-branch first: only needs xT, so the PE keeps
                        # running while the DVE finishes the nmT copy
                        b_ps = p_mm.tile([P, 512], FP32, tag="mm")
                        nc.tensor.matmul(b_ps[:], lhsT=wf_sb[:, 0:2, hs],
                                         rhs=xt_sb[:, 0:2, n0:n0 + 512],
                                         start=True, stop=False, perf_mode=DR)
                        nc.tensor.matmul(b_ps[:], lhsT=wf_sb[:, 2:4, hs],
                                         rhs=xt_sb[:, 2:4, n0:n0 + 512],
                                         start=False, stop=True, perf_mode=DR)
                        nc.vector.tensor_scalar(rb[:, ht, :], b_ps[:],
                                                bf_sb[:, ht:ht + 1], 0.0,
                                                op0=ADD, op1=MAX)

                        a_ps = p_mm.tile([P, 512], FP32, tag="mm")
                        nc.tensor.matmul(a_ps[:], lhsT=wl_sb[:, 0:2, hs],
                                         rhs=nmT[:, 0:2, :],
                                         start=True, stop=False, perf_mode=DR)
                        nc.tensor.matmul(a_ps[:], lhsT=wl_sb[:, 2:4, hs],
                                         rhs=nmT[:, 2:4, :],
                                         start=False, stop=False, perf_mode=DR)
                        nc.tensor.matmul(a_ps[:], lhsT=wr_sb[:, 0:2, hs],
                                         rhs=xt_sb[:, 0:2, n0:n0 + 512],
                                         start=False, stop=False, perf_mode=DR)
                        nc.tensor.matmul(a_ps[:], lhsT=wr_sb[:, 2:4, hs],
                                         rhs=xt_sb[:, 2:4, n0:n0 + 512],
                                         start=False, stop=True, perf_mode=DR)
                        nc.scalar.activation(ra[:, ht, :], a_ps[:], RELU,
                                             bias=bl_sb[:, ht:ht + 1])

                    # per-node readout scalar: s = relu_a.Wout + relu_b.Wout
                    s_ps = p_pool.tile([1, 512], FP32, tag="s")
                    for t in range(4):
                        nc.tensor.matmul(s_ps[:], lhsT=wo_sb[:, :, t:t + 1],
                                         rhs=rb[:, 2 * t:2 * t + 2, :],
                                         start=(t == 0), stop=False, perf_mode=DR)
                    for t in range(4):
                        nc.tensor.matmul(s_ps[:], lhsT=wo_sb[:, :, t:t + 1],
                                         rhs=ra[:, 2 * t:2 * t + 2, :],
                                         start=False, stop=(t == 3), perf_mode=DR)
                    nc.vector.tensor_copy(s_all[0:1, n0:n0 + 512], s_ps[:])

        nc.sync.dma_start(S_OUT[:], s_all[:])

    lower_extended_insts(nc)
    if legalize:
        _legalize_multiwait(nc)
    return nc


def _fp8(a):
    return np.clip(a, -FP8_MAX, FP8_MAX).astype(_NP_FP8)


def _prep(inputs):
    """Host-side sharding/layout prep. Returns (ch_list, in_maps, finish_ctx)."""
    x = np.concatenate(
        [np.asarray(inputs["normal_features"], np.float32),
         np.asarray(inputs["extreme_features"], np.float32)], axis=1)
    src = np.asarray(inputs["edge_index"][0], np.int64)
    dst = np.asarray(inputs["edge_index"][1], np.int64)
    batch = np.asarray(inputs["batch"], np.int64)

    deg = np.bincount(dst, minlength=N_NODES)
    inv = (1.0 / np.maximum(deg, 1)).astype(np.float32)

    # per-core ranking of blocks by descending edge count
    blk = dst // P                                   # global block 0..511
    cnt_b = np.bincount(blk, minlength=N_CORES * NBLK).reshape(N_CORES, NBLK)
    order_kb = np.argsort(-cnt_b, axis=1, kind="stable")   # [core, rank] -> local block
    rank_of_block = np.empty_like(order_kb)
    for k in range(N_CORES):
        rank_of_block[k, order_kb[k]] = np.arange(NBLK)
    cnt_sorted = np.take_along_axis(cnt_b, order_kb, axis=1)   # descending
    ch_list = np.maximum((cnt_sorted + P - 1) // P, 1).max(axis=0)  # [NBLK]
    offs = np.concatenate([[0], np.cumsum(ch_list)]).astype(np.int64)
    NCH = int(offs[-1])

    # sort edges by (core, rank); stable keeps original order within a block
    core_e = blk // NBLK
    rank_e = rank_of_block[core_e, blk % NBLK]
    key = core_e * NBLK + rank_e
    order = np.argsort(key, kind="stable")
    src_s, dst_s, key_s = src[order], dst[order], key[order]
    grp_cnt = np.bincount(key_s, minlength=N_CORES * NBLK)
    grp_start = np.concatenate([[0], np.cumsum(grp_cnt)])
    pos = np.arange(N_EDGES) - grp_start[key_s]
    ki_e = pos % P
    cj_e = offs[key_s % NBLK] + pos // P
    col_e = dst_s % P
    core_start = np.concatenate([[0], np.cumsum(grp_cnt.reshape(N_CORES, NBLK).sum(1))])

    x8 = _fp8(x)
    wl_h = _fp8(np.asarray(inputs["W_l"], np.float32)
                .reshape(4, P, HID).transpose(1, 0, 2))
    wr_h = _fp8(np.asarray(inputs["W_r"], np.float32)
                .reshape(4, P, HID).transpose(1, 0, 2))
    wf_h = _fp8(np.asarray(inputs["W_fc1"], np.float32)
                .reshape(4, P, HID).transpose(1, 0, 2))
    bl_h = np.ascontiguousarray(
        np.asarray(inputs["b_l"], np.float32).reshape(8, P).T)
    bf_h = np.ascontiguousarray(
        np.asarray(inputs["b_fc1"], np.float32).reshape(8, P).T)
    w_out = np.asarray(inputs["W_out"], np.float32).reshape(HID)
    wo_h = np.zeros((P, 2, 16), np.float32)
    for t in range(4):
        for j in range(2):
            wo_h[:, j, t] = w_out[(2 * t + j) * P:(2 * t + j + 1) * P]
    wo_h = _fp8(wo_h)
    idt_h = np.eye(P, dtype=np.float32).astype(ml_dtypes.bfloat16)

    in_maps = []
    node_ids_all = []
    for k in range(N_CORES):
        sl = slice(int(core_start[k]), int(core_start[k + 1]))
        gm = np.zeros((P, NCH, GW), _NP_FP8)
        # scaled edge rows, chunked to bound fp32 temporaries
        ki_k, cj_k, col_k = ki_e[sl], cj_e[sl], col_e[sl]
        src_k, dst_k = src_s[sl], dst_s[sl]
        CHK = 262144
        for c0 in range(0, len(src_k), CHK):
            c1 = min(c0 + CHK, len(src_k))
            rows = x[src_k[c0:c1]] * inv[dst_k[c0:c1]][:, None]
            gm[ki_k[c0:c1], cj_k[c0:c1], 0:D2] = _fp8(rows)
        gm[ki_k, cj_k, D2 + col_k] = 1.0

        # rank-permuted node order for this core
        node_ids = ((k * NBLK + order_kb[k])[:, None] * P
                    + np.arange(P)[None, :]).reshape(-1)
        node_ids_all.append(node_ids)
        xk = x8[node_ids]                                  # [NPC, 512]
        xt_h = np.ascontiguousarray(
            xk.reshape(NPC, 4, P).transpose(2, 1, 0))      # [ki, fs, node]

        in_maps.append({
            "GM": gm, "XT": xt_h,
            "WL": wl_h, "WR": wr_h, "WF": wf_h,
            "BL": bl_h, "BF": bf_h, "WO": wo_h, "IDT": idt_h,
        })

    gcnt = np.bincount(batch, minlength=N_GRAPHS).astype(np.float32)
    finish_ctx = {
        "node_ids": node_ids_all,
        "batch": batch,
        "gcnt": np.maximum(gcnt, 1.0),
        "b_out": np.asarray(inputs["b_out"], np.float32),
    }
    return ch_list, in_maps, finish_ctx


def _finish(s_list, finish_ctx):
    s_glob = np.empty(N_NODES, np.float32)
    for k in range(N_CORES):
        s_glob[finish_ctx["node_ids"][k]] = np.asarray(s_list[k]).reshape(-1)
    sums = np.bincount(finish_ctx["batch"], weights=s_glob,
                       minlength=N_GRAPHS).astype(np.float32)
    logit = sums / finish_ctx["gcnt"] + finish_ctx["b_out"]
    return (1.0 / (1.0 + np.exp(-logit)))[:, None].astype(np.float32)


def _run(inputs, trace=False, sim=False):
    ch_list, in_maps, finish_ctx = _prep(inputs)
    nc = _build_program(ch_list, legalize=not sim)

    if sim:
        from concourse.bass_interp import CoreSim
        csim = CoreSim(nc, require_finite=True, require_nnan=True)
        for name, arr in in_maps[0].items():
            csim.tensor(name)[:] = arr
        csim.simulate(check_with_hw=False)
        return np.array(csim.tensor("s_out")), None

    results = run_bass_kernel_spmd(nc, in_maps, list(range(N_CORES)), trace=trace)
    s_list = [results.results[k]["s_out"] for k in range(N_CORES)]
    return _finish(s_list, finish_ctx), results


def kernel(**inputs) -> np.ndarray:
    out, _ = _run(inputs)
    return out


# revision 10
# speedup vs baseline: 1.0391x; 1.0391x over previous
"""Trainium2 Bass kernel for nn_Discriminator (GNN message passing), v2.

Model (see reference):
    x        = concat(normal, extreme)                     [N, 512]
    neigh    = segment_mean(x[src], dst, N)                [N, 512]
    x_gnn    = relu(neigh @ W_l + b_l + x @ W_r)           [N, 1024]
    x_mlp    = relu(x @ W_fc1 + b_fc1)                     [N, 1024]
    comb     = x_gnn + x_mlp
    gf       = segment_mean(comb, batch, G)                [64, 1024]
    out      = sigmoid(gf @ W_out + b_out)                 [64, 1]

Strategy (v2): nodes sharded by DST across 8 cores (8192 each). The edge
gather x[src] is fully determined by the (host-known) edge index, so the
host pre-expands it: for each core, edges are sorted into per-dst-block
chunks of 128 and the rows x[src]*(1/deg[dst]) are written (fp8 e4m3)
into a dense stream GM[128, NCH, 640] - columns 0:512 hold the scaled
edge features, columns 512:640 hold the one-hot dst-within-block matrix.
The device just streams GM with large sequential DMAs (no gpsimd gather,
no per-edge descriptors) and does the segment-mean as one-hot matmuls in
fp8 DoubleRow mode (K=256 per matmul, 2x feed rate).

Per 128-node block: agg PSUM [node, feat] -> fp8 copy -> 4 PE transposes
-> nmT [feat, node]. Per group of 4 blocks (512 nodes): dense matmuls in
"transposed" orientation out^T[hid, node] with the fp8 weights stationary
and nmT/xT moving (DoubleRow, K=256): a = Wl.nmT + Wr.xT, b = Wf.xT.
Bias+relu fuse into one scalar-engine activation (bias is per-partition
in this orientation). The graph readout contracts hidden with W_out
on-chip: s[node] = relu(a).W_out + relu(b).W_out via M=1 matmuls
(DoubleRow over hid pairs), so only 8192 scalars leave each core. Host
un-permutes s, does the tiny per-graph mean + sigmoid (64 values).

Blocks are processed in per-core descending-edge-count order, with the
per-rank chunk count maxed across cores, so the SPMD program is identical
on all cores while padding stays ~3%.
"""

import numpy as np
import ml_dtypes

import concourse.bass as bass
import concourse.mybir as mybir
import concourse.tile as tile
from concourse.bass_utils import run_bass_kernel_spmd
from concourse.library_overlay import lower_extended_insts

N_NODES = 65536
N_EDGES = 1048576
D2 = 512              # concat feature dim
HID = 1024
N_GRAPHS = 64
N_CORES = 8
NPC = N_NODES // N_CORES      # nodes per core
NBLK = NPC // 128             # 128-node blocks (ranks) per core
P = 128
GW = 640                      # GM row width: 512 feat + 128 one-hot
FP8 = mybir.dt.float8e4
FP32 = mybir.dt.float32
BF16 = mybir.dt.bfloat16
DR = mybir.MatmulPerfMode.DoubleRow

_NP_FP8 = ml_dtypes.float8_e4m3fn
FP8_MAX = 240.0               # TRN fp8e4 saturates at +-240 (not OCP 448)


def _legalize_multiwait(nc):
    """This container's walrus accepts at most one sync-wait per
    instruction; hoist extra waits onto standalone same-engine
    InstEventSemaphore instructions (queues are in-order, so this is
    semantically identical)."""
    n = 0
    for f in nc.m.functions:
        for blk in f.blocks:
            out = []
            changed = False
            for inst in blk.instructions:
                si = getattr(inst, "sync_info", None)
                if si is not None and len(si.on_wait) > 1:
                    waits = list(si.on_wait)
                    for w in waits[:-1]:
                        es = mybir.InstEventSemaphore(
                            name=f"mwz-{inst.name}-{n}", ins=[], outs=[])
                        n += 1
                        es.engine = inst.engine
                        es.sync_info = mybir.SyncInfo(on_wait=[w], on_update=[])
                        out.append(es)
                    inst.sync_info = mybir.SyncInfo(
                        on_wait=[waits[-1]], on_update=list(si.on_update))
                    changed = True
                out.append(inst)
            if changed:
                blk.instructions = out
    return n


def _build_program(ch_list, legalize=True):
    """Build the per-core Bass/Tile program. ch_list[r] = chunk count of
    the rank-r block (identical across cores)."""
    from contextlib import ExitStack

    ch_list = [max(int(c), 1) for c in ch_list]
    CH_MAX = max(ch_list)
    offs = np.concatenate([[0], np.cumsum(ch_list)]).astype(int)
    NCH = int(offs[-1])

    nc = bass.Bass(num_swdge_queues=1)
    GM = nc.declare_dram_parameter("GM", [P, NCH, GW], FP8, isOutput=False)
    XT = nc.declare_dram_parameter("XT", [P, 4, NPC], FP8, isOutput=False)
    WL = nc.declare_dram_parameter("WL", [P, 4, HID], FP8, isOutput=False)
    WR = nc.declare_dram_parameter("WR", [P, 4, HID], FP8, isOutput=False)
    WF = nc.declare_dram_parameter("WF", [P, 4, HID], FP8, isOutput=False)
    BL = nc.declare_dram_parameter("BL", [P, 8], FP32, isOutput=False)
    BF = nc.declare_dram_parameter("BF", [P, 8], FP32, isOutput=False)
    WO = nc.declare_dram_parameter("WO", [P, 2, 16], FP8, isOutput=False)
    IDT = nc.declare_dram_parameter("IDT", [P, P], BF16, isOutput=False)
    S_OUT = nc.declare_dram_parameter("s_out", [1, NPC], FP32, isOutput=True)

    ADD = mybir.AluOpType.add
    MAX = mybir.AluOpType.max
    RELU = mybir.ActivationFunctionType.Relu

    with ExitStack() as ctx:
        tc = ctx.enter_context(tile.TileContext(nc))
        const = ctx.enter_context(tc.tile_pool(name="const", bufs=1))
        gmpool = ctx.enter_context(tc.tile_pool(name="gm", bufs=5))
        nmpool = ctx.enter_context(tc.tile_pool(name="nm", bufs=3))
        ntpool = ctx.enter_context(tc.tile_pool(name="nmT", bufs=2))
        rpool = ctx.enter_context(tc.tile_pool(name="r", bufs=2))
        p_agg = ctx.enter_context(tc.tile_pool(name="pagg", bufs=2, space="PSUM"))
        p_tr = ctx.enter_context(tc.tile_pool(name="ptr", bufs=1, space="PSUM"))
        p_mm = ctx.enter_context(tc.tile_pool(name="pmm", bufs=4, space="PSUM"))
        p_pool = ctx.enter_context(tc.tile_pool(name="ppool", bufs=1, space="PSUM"))

        xt_sb = const.tile([P, 4, NPC], FP8, tag="xt")
        wl_sb = const.tile([P, 4, HID], FP8, tag="wl")
        nc.scalar.dma_start(wl_sb[:], WL[:])
        wr_sb = const.tile([P, 4, HID], FP8, tag="wr")
        nc.scalar.dma_start(wr_sb[:], WR[:])
        wf_sb = const.tile([P, 4, HID], FP8, tag="wf")
        nc.scalar.dma_start(wf_sb[:], WF[:])
        bl_sb = const.tile([P, 8], FP32, tag="bl")
        nc.scalar.dma_start(bl_sb[:], BL[:])
        bf_sb = const.tile([P, 8], FP32, tag="bf")
        nc.scalar.dma_start(bf_sb[:], BF[:])
        wo_sb = const.tile([P, 2, 16], FP8, tag="wo")
        nc.scalar.dma_start(wo_sb[:], WO[:])
        ident = const.tile([P, P], BF16, tag="ident")
        nc.scalar.dma_start(ident[:], IDT[:])
        nc.scalar.dma_start(xt_sb[:, :, 0:2048], XT[:, :, 0:2048])
        s_all = const.tile([1, NPC], FP32, tag="sall")

        nmT = None
        nm_q = []     # blocks aggregated but not yet transposed
        for r in range(NBLK + 1):
            if r < NBLK:
                ch = ch_list[r]
                off = int(offs[r])
                gm = gmpool.tile([P, CH_MAX, GW], FP8, tag="gm")
                eng = nc.sync if r % 2 == 0 else nc.scalar
                eng.dma_start(gm[:, :ch, :], GM[:, off:off + ch, :])
                if r in (5, 7, 9):      # stagger the rest of the xT load
                    q = (r - 3) // 2
                    nc.scalar.dma_start(xt_sb[:, :, q * 2048:(q + 1) * 2048],
                                        XT[:, :, q * 2048:(q + 1) * 2048])

                # segment-sum of pre-scaled edge rows -> neigh mean [node, feat]
                agg = p_agg.tile([P, D2], FP32, tag="agg")
                npairs = ch // 2
                for j in range(npairs):
                    nc.tensor.matmul(
                        agg[:], lhsT=gm[:, 2 * j:2 * j + 2, D2:GW],
                        rhs=gm[:, 2 * j:2 * j + 2, 0:D2],
                        start=(j == 0), stop=(j == npairs - 1 and ch % 2 == 0),
                        perf_mode=DR)
                if ch % 2:
                    nc.tensor.matmul(
                        agg[:], lhsT=gm[:, ch - 1, D2:GW], rhs=gm[:, ch - 1, 0:D2],
                        start=(ch == 1), stop=True)
                nm = nmpool.tile([P, D2], BF16, tag="nm")
                nc.scalar.copy(nm[:], agg[:])
                nm_q.append((r, nm))

            # transposes lag one block so the PSUM->SBUF nm copy hides
            # under the next block's aggregation matmuls
            if len(nm_q) > 1 or r == NBLK:
                r0, nm0 = nm_q.pop(0)
                b = r0 % 4
                if b == 0:
                    nmT = ntpool.tile([P, 4, 512], FP8, tag="nmT")
                # transpose [node, feat] -> [feat, node], bf16 through PSUM
                # (fp8 transpose is rejected by walrus: needs out elem step 2)
                tr = p_tr.tile([P, 4, 256], BF16, tag="tr")
                for fs in range(4):
                    nc.tensor.transpose(tr[:, fs, 0:P],
                                        nm0[:, fs * P:(fs + 1) * P], ident[:])
                nc.vector.tensor_copy(nmT[:, :, b * P:(b + 1) * P], tr[:, :, 0:P])

                if b == 3:
                    g = r0 // 4
                    n0 = g * 512
                    ra = rpool.tile([P, 8, 512], FP8, tag="ra")
                    rb = rpool.tile([P, 8, 512], FP8, tag="rb")
                    # all b-branches first (xT only): covers the nmT
                    # copy latency and finishes relu_b well before the pool
                    for ht in range(8):
                        hs = slice(ht * P, (ht + 1) * P)
                        b_ps = p_mm.tile([P, 512], FP32, tag="mm")
                        nc.tensor.matmul(b_ps[:], lhsT=wf_sb[:, 0:2, hs],
                                         rhs=xt_sb[:, 0:2, n0:n0 + 512],
                                         start=True, stop=False, perf_mode=DR)
                        nc.tensor.matmul(b_ps[:], lhsT=wf_sb[:, 2:4, hs],
                                         rhs=xt_sb[:, 2:4, n0:n0 + 512],
                                         start=False, stop=True, perf_mode=DR)
                        nc.vector.tensor_scalar(rb[:, ht, :], b_ps[:],
                                                bf_sb[:, ht:ht + 1], 0.0,
                                                op0=ADD, op1=MAX)
                    for ht in range(8):
                        hs = slice(ht * P, (ht + 1) * P)
                        a_ps = p_mm.tile([P, 512], FP32, tag="mm")
                        nc.tensor.matmul(a_ps[:], lhsT=wl_sb[:, 0:2, hs],
                                         rhs=nmT[:, 0:2, :],
                                         start=True, stop=False, perf_mode=DR)
                        nc.tensor.matmul(a_ps[:], lhsT=wl_sb[:, 2:4, hs],
                                         rhs=nmT[:, 2:4, :],
                                         start=False, stop=False, perf_mode=DR)
                        nc.tensor.matmul(a_ps[:], lhsT=wr_sb[:, 0:2, hs],
                                         rhs=xt_sb[:, 0:2, n0:n0 + 512],
                                         start=False, stop=False, perf_mode=DR)
                        nc.tensor.matmul(a_ps[:], lhsT=wr_sb[:, 2:4, hs],
                                         rhs=xt_sb[:, 2:4, n0:n0 + 512],
                                         start=False, stop=True, perf_mode=DR)
                        nc.scalar.activation(ra[:, ht, :], a_ps[:], RELU,
                                             bias=bl_sb[:, ht:ht + 1])

                    # per-node readout scalar: s = relu_a.Wout + relu_b.Wout
                    s_ps = p_pool.tile([1, 512], FP32, tag="s")
                    for t in range(4):
                        nc.tensor.matmul(s_ps[:], lhsT=wo_sb[:, :, t:t + 1],
                                         rhs=rb[:, 2 * t:2 * t + 2, :],
                                         start=(t == 0), stop=False, perf_mode=DR)
                    for t in range(4):
                        nc.tensor.matmul(s_ps[:], lhsT=wo_sb[:, :, t:t + 1],
                                         rhs=ra[:, 2 * t:2 * t + 2, :],
                                         start=False, stop=(t == 3), perf_mode=DR)
                    nc.vector.tensor_copy(s_all[0:1, n0:n0 + 512], s_ps[:])
                    if g % 4 == 3:
                        q0 = (g - 3) * 512
                        nc.sync.dma_start(S_OUT[0:1, q0:q0 + 2048],
                                          s_all[0:1, q0:q0 + 2048])

    lower_extended_insts(nc)
    if legalize:
        _legalize_multiwait(nc)
    return nc


def _fp8(a):
    return np.clip(a, -FP8_MAX, FP8_MAX).astype(_NP_FP8)


def _prep(inputs):
    """Host-side sharding/layout prep. Returns (ch_list, in_maps, finish_ctx)."""
    x = np.concatenate(
        [np.asarray(inputs["normal_features"], np.float32),
         np.asarray(inputs["extreme_features"], np.float32)], axis=1)
    src = np.asarray(inputs["edge_index"][0], np.int64)
    dst = np.asarray(inputs["edge_index"][1], np.int64)
    batch = np.asarray(inputs["batch"], np.int64)

    deg = np.bincount(dst, minlength=N_NODES)
    inv = (1.0 / np.maximum(deg, 1)).astype(np.float32)

    # per-core ranking of blocks by descending edge count
    blk = dst // P                                   # global block 0..511
    cnt_b = np.bincount(blk, minlength=N_CORES * NBLK).reshape(N_CORES, NBLK)
    order_kb = np.argsort(-cnt_b, axis=1, kind="stable")   # [core, rank] -> local block
    rank_of_block = np.empty_like(order_kb)
    for k in range(N_CORES):
        rank_of_block[k, order_kb[k]] = np.arange(NBLK)
    cnt_sorted = np.take_along_axis(cnt_b, order_kb, axis=1)   # descending
    ch_list = np.maximum((cnt_sorted + P - 1) // P, 1).max(axis=0)  # [NBLK]
    offs = np.concatenate([[0], np.cumsum(ch_list)]).astype(np.int64)
    NCH = int(offs[-1])

    # sort edges by (core, rank); stable keeps original order within a block
    core_e = blk // NBLK
    rank_e = rank_of_block[core_e, blk % NBLK]
    key = core_e * NBLK + rank_e
    order = np.argsort(key, kind="stable")
    src_s, dst_s, key_s = src[order], dst[order], key[order]
    grp_cnt = np.bincount(key_s, minlength=N_CORES * NBLK)
    grp_start = np.concatenate([[0], np.cumsum(grp_cnt)])
    pos = np.arange(N_EDGES) - grp_start[key_s]
    ki_e = pos % P
    cj_e = offs[key_s % NBLK] + pos // P
    col_e = dst_s % P
    core_start = np.concatenate([[0], np.cumsum(grp_cnt.reshape(N_CORES, NBLK).sum(1))])

    x8 = _fp8(x)
    wl_h = _fp8(np.asarray(inputs["W_l"], np.float32)
                .reshape(4, P, HID).transpose(1, 0, 2))
    wr_h = _fp8(np.asarray(inputs["W_r"], np.float32)
                .reshape(4, P, HID).transpose(1, 0, 2))
    wf_h = _fp8(np.asarray(inputs["W_fc1"], np.float32)
                .reshape(4, P, HID).transpose(1, 0, 2))
    bl_h = np.ascontiguousarray(
        np.asarray(inputs["b_l"], np.float32).reshape(8, P).T)
    bf_h = np.ascontiguousarray(
        np.asarray(inputs["b_fc1"], np.float32).reshape(8, P).T)
    w_out = np.asarray(inputs["W_out"], np.float32).reshape(HID)
    wo_h = np.zeros((P, 2, 16), np.float32)
    for t in range(4):
        for j in range(2):
            wo_h[:, j, t] = w_out[(2 * t + j) * P:(2 * t + j + 1) * P]
    wo_h = _fp8(wo_h)
    idt_h = np.eye(P, dtype=np.float32).astype(ml_dtypes.bfloat16)

    in_maps = []
    node_ids_all = []
    for k in range(N_CORES):
        sl = slice(int(core_start[k]), int(core_start[k + 1]))
        gm = np.zeros((P, NCH, GW), _NP_FP8)
        # scaled edge rows, chunked to bound fp32 temporaries
        ki_k, cj_k, col_k = ki_e[sl], cj_e[sl], col_e[sl]
        src_k, dst_k = src_s[sl], dst_s[sl]
        CHK = 262144
        for c0 in range(0, len(src_k), CHK):
            c1 = min(c0 + CHK, len(src_k))
            rows = x[src_k[c0:c1]] * inv[dst_k[c0:c1]][:, None]
            gm[ki_k[c0:c1], cj_k[c0:c1], 0:D2] = _fp8(rows)
        gm[ki_k, cj_k, D2 + col_k] = 1.0

        # rank-permuted node order for this core
        node_ids = ((k * NBLK + order_kb[k])[:, None] * P
                    + np.arange(P)[None, :]).reshape(-1)
        node_ids_all.append(node_ids)
        xk = x8[node_ids]                                  # [NPC, 512]
        xt_h = np.ascontiguousarray(
            xk.reshape(NPC, 4, P).transpose(2, 1, 0))      # [ki, fs, node]

        in_maps.append({
            "GM": gm, "XT": xt_h,
            "WL": wl_h, "WR": wr_h, "WF": wf_h,
            "BL": bl_h, "BF": bf_h, "WO": wo_h, "IDT": idt_h,
        })

    gcnt = np.bincount(batch, minlength=N_GRAPHS).astype(np.float32)
    finish_ctx = {
        "node_ids": node_ids_all,
        "batch": batch,
        "gcnt": np.maximum(gcnt, 1.0),
        "b_out": np.asarray(inputs["b_out"], np.float32),
    }
    return ch_list, in_maps, finish_ctx


def _finish(s_list, finish_ctx):
    s_glob = np.empty(N_NODES, np.float32)
    for k in range(N_CORES):
        s_glob[finish_ctx["node_ids"][k]] = np.asarray(s_list[k]).reshape(-1)
    sums = np.bincount(finish_ctx["batch"], weights=s_glob,
                       minlength=N_GRAPHS).astype(np.float32)
    logit = sums / finish_ctx["gcnt"] + finish_ctx["b_out"]
    return (1.0 / (1.0 + np.exp(-logit)))[:, None].astype(np.float32)


def _run(inputs, trace=False, sim=False):
    ch_list, in_maps, finish_ctx = _prep(inputs)
    nc = _build_program(ch_list, legalize=not sim)

    if sim:
        from concourse.bass_interp import CoreSim
        csim = CoreSim(nc, require_finite=True, require_nnan=True)
        for name, arr in in_maps[0].items():
            csim.tensor(name)[:] = arr
        csim.simulate(check_with_hw=False)
        return np.array(csim.tensor("s_out")), None

    results = run_bass_kernel_spmd(nc, in_maps, list(range(N_CORES)), trace=trace)
    s_list = [results.results[k]["s_out"] for k in range(N_CORES)]
    return _finish(s_list, finish_ctx), results


def kernel(**inputs) -> np.ndarray:
    out, _ = _run(inputs)
    return out


# revision 12
# speedup vs baseline: 1.0418x; 1.0026x over previous
"""Trainium2 Bass kernel for nn_Discriminator (GNN message passing), v2.

Model (see reference):
    x        = concat(normal, extreme)                     [N, 512]
    neigh    = segment_mean(x[src], dst, N)                [N, 512]
    x_gnn    = relu(neigh @ W_l + b_l + x @ W_r)           [N, 1024]
    x_mlp    = relu(x @ W_fc1 + b_fc1)                     [N, 1024]
    comb     = x_gnn + x_mlp
    gf       = segment_mean(comb, batch, G)                [64, 1024]
    out      = sigmoid(gf @ W_out + b_out)                 [64, 1]

Strategy (v2): nodes sharded by DST across 8 cores (8192 each). The edge
gather x[src] is fully determined by the (host-known) edge index, so the
host pre-expands it: for each core, edges are sorted into per-dst-block
chunks of 128 and the rows x[src]*(1/deg[dst]) are written (fp8 e4m3)
into a dense stream GM[128, NCH, 640] - columns 0:512 hold the scaled
edge features, columns 512:640 hold the one-hot dst-within-block matrix.
The device just streams GM with large sequential DMAs (no gpsimd gather,
no per-edge descriptors) and does the segment-mean as one-hot matmuls in
fp8 DoubleRow mode (K=256 per matmul, 2x feed rate).

Per 128-node block: agg PSUM [node, feat] -> fp8 copy -> 4 PE transposes
-> nmT [feat, node]. Per group of 4 blocks (512 nodes): dense matmuls in
"transposed" orientation out^T[hid, node] with the fp8 weights stationary
and nmT/xT moving (DoubleRow, K=256): a = Wl.nmT + Wr.xT, b = Wf.xT.
Bias+relu fuse into one scalar-engine activation (bias is per-partition
in this orientation). The graph readout contracts hidden with W_out
on-chip: s[node] = relu(a).W_out + relu(b).W_out via M=1 matmuls
(DoubleRow over hid pairs), so only 8192 scalars leave each core. Host
un-permutes s, does the tiny per-graph mean + sigmoid (64 values).

Blocks are processed in per-core descending-edge-count order, with the
per-rank chunk count maxed across cores, so the SPMD program is identical
on all cores while padding stays ~3%.
"""

import numpy as np
import ml_dtypes

import concourse.bass as bass
import concourse.mybir as mybir
import concourse.tile as tile
from concourse.bass_utils import run_bass_kernel_spmd
from concourse.library_overlay import lower_extended_insts

N_NODES = 65536
N_EDGES = 1048576
D2 = 512              # concat feature dim
HID = 1024
N_GRAPHS = 64
N_CORES = 8
NPC = N_NODES // N_CORES      # nodes per core
NBLK = NPC // 128             # 128-node blocks (ranks) per core
P = 128
GW = 640                      # GM row width: 512 feat + 128 one-hot
FP8 = mybir.dt.float8e4
FP32 = mybir.dt.float32
BF16 = mybir.dt.bfloat16
DR = mybir.MatmulPerfMode.DoubleRow

_NP_FP8 = ml_dtypes.float8_e4m3fn
FP8_MAX = 240.0               # TRN fp8e4 saturates at +-240 (not OCP 448)


def _legalize_multiwait(nc):
    """This container's walrus accepts at most one sync-wait per
    instruction; hoist extra waits onto standalone same-engine
    InstEventSemaphore instructions (queues are in-order, so this is
    semantically identical)."""
    n = 0
    for f in nc.m.functions:
        for blk in f.blocks:
            out = []
            changed = False
            for inst in blk.instructions:
                si = getattr(inst, "sync_info", None)
                if si is not None and len(si.on_wait) > 1:
                    waits = list(si.on_wait)
                    for w in waits[:-1]:
                        es = mybir.InstEventSemaphore(
                            name=f"mwz-{inst.name}-{n}", ins=[], outs=[])
                        n += 1
                        es.engine = inst.engine
                        es.sync_info = mybir.SyncInfo(on_wait=[w], on_update=[])
                        out.append(es)
                    inst.sync_info = mybir.SyncInfo(
                        on_wait=[waits[-1]], on_update=list(si.on_update))
                    changed = True
                out.append(inst)
            if changed:
                blk.instructions = out
    return n


def _build_program(ch_list, legalize=True):
    """Build the per-core Bass/Tile program. ch_list[r] = chunk count of
    the rank-r block (identical across cores)."""
    from contextlib import ExitStack

    ch_list = [max(int(c), 1) for c in ch_list]
    CH_MAX = max(ch_list)
    offs = np.concatenate([[0], np.cumsum(ch_list)]).astype(int)
    NCH = int(offs[-1])

    nc = bass.Bass(num_swdge_queues=1)
    GM = nc.declare_dram_parameter("GM", [P, NCH, GW], FP8, isOutput=False)
    XT = nc.declare_dram_parameter("XT", [P, 4, NPC], FP8, isOutput=False)
    WL = nc.declare_dram_parameter("WL", [P, 4, HID], FP8, isOutput=False)
    WR = nc.declare_dram_parameter("WR", [P, 4, HID], FP8, isOutput=False)
    WF = nc.declare_dram_parameter("WF", [P, 4, HID], FP8, isOutput=False)
    BL = nc.declare_dram_parameter("BL", [P, 8], FP32, isOutput=False)
    BF = nc.declare_dram_parameter("BF", [P, 8], FP32, isOutput=False)
    WO = nc.declare_dram_parameter("WO", [P, 2, 16], FP8, isOutput=False)
    IDT = nc.declare_dram_parameter("IDT", [P, P], BF16, isOutput=False)
    S_OUT = nc.declare_dram_parameter("s_out", [1, NPC], FP32, isOutput=True)

    ADD = mybir.AluOpType.add
    MAX = mybir.AluOpType.max
    RELU = mybir.ActivationFunctionType.Relu

    with ExitStack() as ctx:
        tc = ctx.enter_context(tile.TileContext(nc))
        const = ctx.enter_context(tc.tile_pool(name="const", bufs=1))
        gmpool = ctx.enter_context(tc.tile_pool(name="gm", bufs=5))
        nmpool = ctx.enter_context(tc.tile_pool(name="nm", bufs=3))
        ntpool = ctx.enter_context(tc.tile_pool(name="nmT", bufs=2))
        rpool = ctx.enter_context(tc.tile_pool(name="r", bufs=2))
        p_agg = ctx.enter_context(tc.tile_pool(name="pagg", bufs=2, space="PSUM"))
        p_tr = ctx.enter_context(tc.tile_pool(name="ptr", bufs=1, space="PSUM"))
        p_mm = ctx.enter_context(tc.tile_pool(name="pmm", bufs=4, space="PSUM"))
        p_pool = ctx.enter_context(tc.tile_pool(name="ppool", bufs=1, space="PSUM"))

        xt_sb = const.tile([P, 4, NPC], FP8, tag="xt")
        wl_sb = const.tile([P, 4, HID], FP8, tag="wl")
        wr_sb = const.tile([P, 4, HID], FP8, tag="wr")
        wf_sb = const.tile([P, 4, HID], FP8, tag="wf")
        bl_sb = const.tile([P, 8], FP32, tag="bl")
        nc.scalar.dma_start(bl_sb[:], BL[:])
        bf_sb = const.tile([P, 8], FP32, tag="bf")
        nc.scalar.dma_start(bf_sb[:], BF[:])
        wo_sb = const.tile([P, 2, 16], FP8, tag="wo")
        nc.scalar.dma_start(wo_sb[:], WO[:])
        ident = const.tile([P, P], BF16, tag="ident")
        nc.scalar.dma_start(ident[:], IDT[:])
        s_all = const.tile([1, NPC], FP32, tag="sall")

        nmT = None
        nm_q = []     # blocks aggregated but not yet transposed
        for r in range(NBLK + 1):
            if r < NBLK:
                ch = ch_list[r]
                off = int(offs[r])
                gm = gmpool.tile([P, CH_MAX, GW], FP8, tag="gm")
                eng = nc.sync if r % 2 == 0 else nc.scalar
                # half-split so aggregation starts on the first half
                h1 = min(2 * ((ch // 2 + 1) // 2), ch)  # even #chunks in first half
                eng.dma_start(gm[:, :h1, :], GM[:, off:off + h1, :])
                if h1 < ch:
                    eng.dma_start(gm[:, h1:ch, :], GM[:, off + h1:off + ch, :])
                if r == 2:              # weights + first xT quarter after gm(1)
                    nc.scalar.dma_start(wl_sb[:], WL[:])
                    nc.scalar.dma_start(wr_sb[:], WR[:])
                    nc.scalar.dma_start(wf_sb[:], WF[:])
                    nc.scalar.dma_start(xt_sb[:, :, 0:2048], XT[:, :, 0:2048])
                if r in (5, 7, 9):      # stagger the rest of the xT load
                    q = (r - 3) // 2
                    nc.scalar.dma_start(xt_sb[:, :, q * 2048:(q + 1) * 2048],
                                        XT[:, :, q * 2048:(q + 1) * 2048])

                # segment-sum of pre-scaled edge rows -> neigh mean [node, feat]
                agg = p_agg.tile([P, D2], FP32, tag="agg")
                npairs = ch // 2
                for j in range(npairs):
                    nc.tensor.matmul(
                        agg[:], lhsT=gm[:, 2 * j:2 * j + 2, D2:GW],
                        rhs=gm[:, 2 * j:2 * j + 2, 0:D2],
                        start=(j == 0), stop=(j == npairs - 1 and ch % 2 == 0),
                        perf_mode=DR)
                if ch % 2:
                    nc.tensor.matmul(
                        agg[:], lhsT=gm[:, ch - 1, D2:GW], rhs=gm[:, ch - 1, 0:D2],
                        start=(ch == 1), stop=True)
                nm = nmpool.tile([P, D2], BF16, tag="nm")
                nc.scalar.copy(nm[:], agg[:])
                nm_q.append((r, nm))

            # transposes lag one block so the PSUM->SBUF nm copy hides
            # under the next block's aggregation matmuls
            if len(nm_q) > 1 or r == NBLK:
                r0, nm0 = nm_q.pop(0)
                b = r0 % 4
                if b == 0:
                    nmT = ntpool.tile([P, 4, 512], FP8, tag="nmT")
                # transpose [node, feat] -> [feat, node], bf16 through PSUM
                # (fp8 transpose is rejected by walrus: needs out elem step 2)
                tr = p_tr.tile([P, 4, 256], BF16, tag="tr")
                for fs in range(4):
                    nc.tensor.transpose(tr[:, fs, 0:P],
                                        nm0[:, fs * P:(fs + 1) * P], ident[:])
                nc.vector.tensor_copy(nmT[:, :, b * P:(b + 1) * P], tr[:, :, 0:P])

                if b == 3:
                    g = r0 // 4
                    n0 = g * 512
                    ra = rpool.tile([P, 8, 512], FP8, tag="ra")
                    rb = rpool.tile([P, 8, 512], FP8, tag="rb")
                    # all b-branches first (xT only): covers the nmT
                    # copy latency and finishes relu_b well before the pool
                    for ht in range(8):
                        hs = slice(ht * P, (ht + 1) * P)
                        b_ps = p_mm.tile([P, 512], FP32, tag="mm")
                        nc.tensor.matmul(b_ps[:], lhsT=wf_sb[:, 0:2, hs],
                                         rhs=xt_sb[:, 0:2, n0:n0 + 512],
                                         start=True, stop=False, perf_mode=DR)
                        nc.tensor.matmul(b_ps[:], lhsT=wf_sb[:, 2:4, hs],
                                         rhs=xt_sb[:, 2:4, n0:n0 + 512],
                                         start=False, stop=True, perf_mode=DR)
                        nc.vector.tensor_scalar(rb[:, ht, :], b_ps[:],
                                                bf_sb[:, ht:ht + 1], 0.0,
                                                op0=ADD, op1=MAX)
                    s_ps = p_pool.tile([1, 512], FP32, tag="s")
                    for ht in range(8):
                        hs = slice(ht * P, (ht + 1) * P)
                        a_ps = p_mm.tile([P, 512], FP32, tag="mm")
                        nc.tensor.matmul(a_ps[:], lhsT=wl_sb[:, 0:2, hs],
                                         rhs=nmT[:, 0:2, :],
                                         start=True, stop=False, perf_mode=DR)
                        nc.tensor.matmul(a_ps[:], lhsT=wl_sb[:, 2:4, hs],
                                         rhs=nmT[:, 2:4, :],
                                         start=False, stop=False, perf_mode=DR)
                        nc.tensor.matmul(a_ps[:], lhsT=wr_sb[:, 0:2, hs],
                                         rhs=xt_sb[:, 0:2, n0:n0 + 512],
                                         start=False, stop=False, perf_mode=DR)
                        nc.tensor.matmul(a_ps[:], lhsT=wr_sb[:, 2:4, hs],
                                         rhs=xt_sb[:, 2:4, n0:n0 + 512],
                                         start=False, stop=True, perf_mode=DR)
                        nc.scalar.activation(ra[:, ht, :], a_ps[:], RELU,
                                             bias=bl_sb[:, ht:ht + 1])
                        # interleave the rb-readout so it never tail-stalls
                        if ht % 2 == 1:
                            t = ht // 2
                            nc.tensor.matmul(s_ps[:], lhsT=wo_sb[:, :, t:t + 1],
                                             rhs=rb[:, 2 * t:2 * t + 2, :],
                                             start=(t == 0), stop=False,
                                             perf_mode=DR)
                    for t in range(4):
                        nc.tensor.matmul(s_ps[:], lhsT=wo_sb[:, :, t:t + 1],
                                         rhs=ra[:, 2 * t:2 * t + 2, :],
                                         start=False, stop=(t == 3), perf_mode=DR)
                    nc.vector.tensor_copy(s_all[0:1, n0:n0 + 512], s_ps[:])
                    if g % 4 == 3:
                        q0 = (g - 3) * 512
                        nc.sync.dma_start(S_OUT[0:1, q0:q0 + 2048],
                                          s_all[0:1, q0:q0 + 2048])

    lower_extended_insts(nc)
    if legalize:
        _legalize_multiwait(nc)
    return nc


def _fp8(a):
    return np.clip(a, -FP8_MAX, FP8_MAX).astype(_NP_FP8)


def _prep(inputs):
    """Host-side sharding/layout prep. Returns (ch_list, in_maps, finish_ctx)."""
    x = np.concatenate(
        [np.asarray(inputs["normal_features"], np.float32),
         np.asarray(inputs["extreme_features"], np.float32)], axis=1)
    src = np.asarray(inputs["edge_index"][0], np.int64)
    dst = np.asarray(inputs["edge_index"][1], np.int64)
    batch = np.asarray(inputs["batch"], np.int64)

    deg = np.bincount(dst, minlength=N_NODES)
    inv = (1.0 / np.maximum(deg, 1)).astype(np.float32)

    # per-core ranking of blocks by descending edge count
    blk = dst // P                                   # global block 0..511
    cnt_b = np.bincount(blk, minlength=N_CORES * NBLK).reshape(N_CORES, NBLK)
    order_kb = np.argsort(-cnt_b, axis=1, kind="stable")   # [core, rank] -> local block
    rank_of_block = np.empty_like(order_kb)
    for k in range(N_CORES):
        rank_of_block[k, order_kb[k]] = np.arange(NBLK)
    cnt_sorted = np.take_along_axis(cnt_b, order_kb, axis=1)   # descending
    ch_list = np.maximum((cnt_sorted + P - 1) // P, 1).max(axis=0)  # [NBLK]
    offs = np.concatenate([[0], np.cumsum(ch_list)]).astype(np.int64)
    NCH = int(offs[-1])

    # sort edges by (core, rank); stable keeps original order within a block
    core_e = blk // NBLK
    rank_e = rank_of_block[core_e, blk % NBLK]
    key = core_e * NBLK + rank_e
    order = np.argsort(key, kind="stable")
    src_s, dst_s, key_s = src[order], dst[order], key[order]
    grp_cnt = np.bincount(key_s, minlength=N_CORES * NBLK)
    grp_start = np.concatenate([[0], np.cumsum(grp_cnt)])
    pos = np.arange(N_EDGES) - grp_start[key_s]
    ki_e = pos % P
    cj_e = offs[key_s % NBLK] + pos // P
    col_e = dst_s % P
    core_start = np.concatenate([[0], np.cumsum(grp_cnt.reshape(N_CORES, NBLK).sum(1))])

    x8 = _fp8(x)
    wl_h = _fp8(np.asarray(inputs["W_l"], np.float32)
                .reshape(4, P, HID).transpose(1, 0, 2))
    wr_h = _fp8(np.asarray(inputs["W_r"], np.float32)
                .reshape(4, P, HID).transpose(1, 0, 2))
    wf_h = _fp8(np.asarray(inputs["W_fc1"], np.float32)
                .reshape(4, P, HID).transpose(1, 0, 2))
    bl_h = np.ascontiguousarray(
        np.asarray(inputs["b_l"], np.float32).reshape(8, P).T)
    bf_h = np.ascontiguousarray(
        np.asarray(inputs["b_fc1"], np.float32).reshape(8, P).T)
    w_out = np.asarray(inputs["W_out"], np.float32).reshape(HID)
    wo_h = np.zeros((P, 2, 16), np.float32)
    for t in range(4):
        for j in range(2):
            wo_h[:, j, t] = w_out[(2 * t + j) * P:(2 * t + j + 1) * P]
    wo_h = _fp8(wo_h)
    idt_h = np.eye(P, dtype=np.float32).astype(ml_dtypes.bfloat16)

    in_maps = []
    node_ids_all = []
    for k in range(N_CORES):
        sl = slice(int(core_start[k]), int(core_start[k + 1]))
        gm = np.zeros((P, NCH, GW), _NP_FP8)
        # scaled edge rows, chunked to bound fp32 temporaries
        ki_k, cj_k, col_k = ki_e[sl], cj_e[sl], col_e[sl]
        src_k, dst_k = src_s[sl], dst_s[sl]
        CHK = 262144
        for c0 in range(0, len(src_k), CHK):
            c1 = min(c0 + CHK, len(src_k))
            rows = x[src_k[c0:c1]] * inv[dst_k[c0:c1]][:, None]
            gm[ki_k[c0:c1], cj_k[c0:c1], 0:D2] = _fp8(rows)
        gm[ki_k, cj_k, D2 + col_k] = 1.0

        # rank-permuted node order for this core
        node_ids = ((k * NBLK + order_kb[k])[:, None] * P
                    + np.arange(P)[None, :]).reshape(-1)
        node_ids_all.append(node_ids)
        xk = x8[node_ids]                                  # [NPC, 512]
        xt_h = np.ascontiguousarray(
            xk.reshape(NPC, 4, P).transpose(2, 1, 0))      # [ki, fs, node]

        in_maps.append({
            "GM": gm, "XT": xt_h,
            "WL": wl_h, "WR": wr_h, "WF": wf_h,
            "BL": bl_h, "BF": bf_h, "WO": wo_h, "IDT": idt_h,
        })

    gcnt = np.bincount(batch, minlength=N_GRAPHS).astype(np.float32)
    finish_ctx = {
        "node_ids": node_ids_all,
        "batch": batch,
        "gcnt": np.maximum(gcnt, 1.0),
        "b_out": np.asarray(inputs["b_out"], np.float32),
    }
    return ch_list, in_maps, finish_ctx


def _finish(s_list, finish_ctx):
    s_glob = np.empty(N_NODES, np.float32)
    for k in range(N_CORES):
        s_glob[finish_ctx["node_ids"][k]] = np.asarray(s_list[k]).reshape(-1)
    sums = np.bincount(finish_ctx["batch"], weights=s_glob,
                       minlength=N_GRAPHS).astype(np.float32)
    logit = sums / finish_ctx["gcnt"] + finish_ctx["b_out"]
    return (1.0 / (1.0 + np.exp(-logit)))[:, None].astype(np.float32)


def _run(inputs, trace=False, sim=False):
    ch_list, in_maps, finish_ctx = _prep(inputs)
    nc = _build_program(ch_list, legalize=not sim)

    if sim:
        from concourse.bass_interp import CoreSim
        csim = CoreSim(nc, require_finite=True, require_nnan=True)
        for name, arr in in_maps[0].items():
            csim.tensor(name)[:] = arr
        csim.simulate(check_with_hw=False)
        return np.array(csim.tensor("s_out")), None

    results = run_bass_kernel_spmd(nc, in_maps, list(range(N_CORES)), trace=trace)
    s_list = [results.results[k]["s_out"] for k in range(N_CORES)]
    return _finish(s_list, finish_ctx), results


def kernel(**inputs) -> np.ndarray:
    out, _ = _run(inputs)
    return out
